# revision 6
# baseline (speedup 1.0000x reference)
"""Trainium2 Bass kernel for nn_DynamicKnowledgeInjector.

Reference computation (per batch b, token t):
    rel_mask = surviving_mask[..., f_i] & surviving_mask[..., f_j]   [B,T,R]
    ta = rel_embs @ Wt.T + bt                                        [R,H]
    Q  = qh @ Wq.T + bq ;  K = ta @ Wk.T + bk ;  V = ta @ Wv.T + bv
    scores = Q @ K.T / sqrt(H), masked to -inf where !rel_mask
    top-28 sparsify -> softmax -> out = attn @ V  (zero row if no active rel)

Sharding: data-parallel over batch; core c owns batch c. Relation-table
work (ta/K/V) is replicated on every core; no collectives.

Device layouts (all activations [feature, token]):
    qhT[H,T], QT[H,T], taT[H,R], KT[H,R] ; V kept natural [R,H] (bf16)
    scores computed [t, r] per 128-token tile, top-k along free dim,
    attn PE-transposed to [r, t] tiles for the AV matmul.

Masking: scores matmul gets a 9th contraction tile of 65 rows:
    lhsT rows = [surviving_mask.T (0/1) ; ones] for the token tile,
    rhs  rows = [BIG*(onehot(f_i)+onehot(f_j)) ; -2*BIG * ones]
so the accumulated bias is BIG*(m_i+m_j-2): exactly 0 for active pairs
(16384+16384-32768 is exact in fp32), -BIG or -2*BIG otherwise. exp()
then underflows those to exactly 0, matching the -inf reference.
"""

import math

import numpy as np

B, T, H, E, F, TOP_K = 8, 2048, 1024, 768, 64, 28
R = 2016
P = 128
BIG = 16384.0  # power of two: mask bias arithmetic is exact in fp32
NEG_HUGE = -1.0e30

N_CORES = 8
HT = H // P   # 8  h-tiles
ET = E // P   # 6  e-tiles
TT = T // P   # 16 t-tiles
# r split into column slices that each fit one PSUM bank (<=512 fp32)
R_SLICES = [(0, 512), (512, 512), (1024, 512), (1536, 480)]
# r split into 128-row contraction tiles for the AV matmul (last is 96)
RT = (R + P - 1) // P  # 16
R_TILES = [(i * P, min(P, R - i * P)) for i in range(RT)]

_CACHE = {}


def _build_program():
    import concourse.bass as bass
    import concourse.mybir as mybir
    from contextlib import ExitStack
    from concourse.tile import TileContext
    from concourse.masks import make_identity

    fp32 = mybir.dt.float32
    bf16 = mybir.dt.bfloat16

    nc = bass.Bass()

    # ---------------- DRAM parameters ----------------
    qhT_d = nc.declare_dram_parameter("qhT", [H, T], fp32, isOutput=False)
    smf1T_d = nc.declare_dram_parameter("smf1T", [F + 1, T], fp32, isOutput=False)
    maskrhs_d = nc.declare_dram_parameter("maskrhs", [F + 1, R], fp32, isOutput=False)
    relT_d = nc.declare_dram_parameter("relT", [E, R], fp32, isOutput=False)
    WtT_d = nc.declare_dram_parameter("WtT", [E, H], fp32, isOutput=False)
    WkT_d = nc.declare_dram_parameter("WkT", [H, H], fp32, isOutput=False)
    WvT_d = nc.declare_dram_parameter("WvT", [H, H], fp32, isOutput=False)
    WqTs_d = nc.declare_dram_parameter("WqTs", [H, H], fp32, isOutput=False)
    bt_d = nc.declare_dram_parameter("bt", [H], fp32, isOutput=False)
    bk_d = nc.declare_dram_parameter("bk", [H], fp32, isOutput=False)
    bv_d = nc.declare_dram_parameter("bv", [H], fp32, isOutput=False)
    bqs_d = nc.declare_dram_parameter("bqs", [H], fp32, isOutput=False)
    out_d = nc.declare_dram_parameter("out", [T, H], fp32, isOutput=True)

    def part_tiles(ap_2d, p=P):
        # [A*p, N] dram view -> [p, A, N] (partition-major tiling of rows)
        return ap_2d.rearrange("(a p) n -> p a n", p=p)

    with TileContext(nc) as tc, ExitStack() as ctx:
        # ------------- resident pools -------------
        res_pool = ctx.enter_context(tc.tile_pool(name="resident", bufs=1))
        KT_sb = res_pool.tile([P, HT, R], fp32, tag="KT")      # [h_loc, ht*R + r] 8MB
        V_sb = res_pool.tile([P, RT, H], bf16, tag="V")        # [r_loc, rt*H + h] 4MB
        smf1T_sb = res_pool.tile([F + 1, T], fp32, tag="smf")
        maskrhs_sb = res_pool.tile([F + 1, R], fp32, tag="mrhs")
        ident_sb = res_pool.tile([P, P], fp32, tag="ident")
        ones1_sb = res_pool.tile([1, P], fp32, tag="ones1")
        bv_sb = res_pool.tile([1, H], fp32, tag="bv")
        bias_sb = res_pool.tile([P, 3 * HT], fp32, tag="biases")  # bt|bk|bqs

        nc.sync.dma_start(smf1T_sb[:], smf1T_d[:])
        nc.sync.dma_start(maskrhs_sb[:], maskrhs_d[:])
        nc.sync.dma_start(bv_sb[:], bv_d[None, :])
        nc.sync.dma_start(bias_sb[:, 0:HT], bt_d[:].rearrange("(a p) -> p a", p=P))
        nc.sync.dma_start(bias_sb[:, HT:2 * HT], bk_d[:].rearrange("(a p) -> p a", p=P))
        nc.sync.dma_start(bias_sb[:, 2 * HT:3 * HT], bqs_d[:].rearrange("(a p) -> p a", p=P))
        make_identity(nc, ident_sb[:])
        nc.vector.memset(ones1_sb[:], 1.0)
        bt_sl = lambda m: bias_sb[:, m:m + 1]
        bk_sl = lambda m: bias_sb[:, HT + m:HT + m + 1]
        bq_sl = lambda m: bias_sb[:, 2 * HT + m:2 * HT + m + 1]

        # internal DRAM spill for taT and QT
        dram_pool = ctx.enter_context(tc.tile_pool(name="dram", bufs=1, space="DRAM"))
        taT_dd = dram_pool.tile([H, R], fp32, tag="taT")
        QT_dd = dram_pool.tile([H, T], fp32, tag="QT")

        # ================= phase A: taT[h', r] = (rel_embs @ Wt.T + bt).T =================
        with ExitStack() as actx:
            aw = actx.enter_context(tc.tile_pool(name="ph_a", bufs=1))
            WtT_sb = aw.tile([P, ET, H], fp32, tag="WtT")
            relT_sb = aw.tile([P, ET, R], fp32, tag="relT")
            nc.sync.dma_start(WtT_sb[:], part_tiles(WtT_d[:]))
            nc.sync.dma_start(relT_sb[:], part_tiles(relT_d[:]))
            aps = actx.enter_context(tc.tile_pool(name="ph_a_ps", bufs=4, space="PSUM"))
            abuf = actx.enter_context(tc.tile_pool(name="ph_a_buf", bufs=4))
            for m in range(HT):
                for (r0, rw) in R_SLICES:
                    ps = aps.tile([P, 512], fp32, tag="ps")
                    for k in range(ET):
                        nc.tensor.matmul(
                            ps[:, 0:rw],
                            WtT_sb[:, k, m * P:(m + 1) * P],
                            relT_sb[:, k, r0:r0 + rw],
                            start=(k == 0), stop=(k == ET - 1),
                        )
                    buf = abuf.tile([P, 512], fp32, tag="buf")
                    nc.scalar.activation(buf[:, 0:rw], ps[:, 0:rw],
                                         mybir.ActivationFunctionType.Identity,
                                         bias=bt_sl(m))
                    nc.sync.dma_start(taT_dd[m * P:(m + 1) * P, r0:r0 + rw], buf[:, 0:rw])

        # ================= phase B: KT[h, r] = (ta @ Wk.T + bk).T =================
        with ExitStack() as bctx:
            bw = bctx.enter_context(tc.tile_pool(name="ph_b", bufs=1))
            WkT_sb = bw.tile([P, HT, H], fp32, tag="WkT")
            nc.sync.dma_start(WkT_sb[:], part_tiles(WkT_d[:]))
            bch = bctx.enter_context(tc.tile_pool(name="ph_b_ch", bufs=2))
            bps = bctx.enter_context(tc.tile_pool(name="ph_b_ps", bufs=4, space="PSUM"))
            for (r0, rw) in R_SLICES:
                ch = bch.tile([P, HT, 512], fp32, tag="tach")
                nc.sync.dma_start(
                    ch[:, :, 0:rw],
                    taT_dd[:, r0:r0 + rw].rearrange("(a p) r -> p a r", p=P))
                for m in range(HT):
                    ps = bps.tile([P, 512], fp32, tag="ps")
                    for k in range(HT):
                        nc.tensor.matmul(
                            ps[:, 0:rw],
                            WkT_sb[:, k, m * P:(m + 1) * P],
                            ch[:, k, 0:rw],
                            start=(k == 0), stop=(k == HT - 1),
                        )
                    nc.scalar.activation(KT_sb[:, m, r0:r0 + rw],
                                         ps[:, 0:rw],
                                         mybir.ActivationFunctionType.Identity,
                                         bias=bk_sl(m))

        # ================= phase C: V[r, h] = ta @ Wv.T + bv (bf16) =================
        with ExitStack() as cctx:
            cw = cctx.enter_context(tc.tile_pool(name="ph_c", bufs=1))
            WvT_sb = cw.tile([P, HT, H], fp32, tag="WvT")
            nc.sync.dma_start(WvT_sb[:], part_tiles(WvT_d[:]))
            cch = cctx.enter_context(tc.tile_pool(name="ph_c_ch", bufs=2))
            cps = cctx.enter_context(tc.tile_pool(name="ph_c_ps", bufs=2, space="PSUM"))
            for ri, (r0, rw) in enumerate(R_SLICES):
                ch = cch.tile([P, HT, 512], fp32, tag="tach")
                nc.sync.dma_start(
                    ch[:, :, 0:rw],
                    taT_dd[:, r0:r0 + rw].rearrange("(a p) r -> p a r", p=P))
                for rm in range((rw + P - 1) // P):
                    g = ri * 4 + rm            # global 128-row r-tile index
                    rws = min(P, rw - rm * P)  # 128 or 96
                    ps = cps.tile([P, H], fp32, tag="ps")
                    for hn in range(2):
                        for k in range(HT):
                            nc.tensor.matmul(
                                ps[0:rws, hn * 512:(hn + 1) * 512],
                                ch[:, k, rm * P: rm * P + rws],
                                WvT_sb[:, k, hn * 512:(hn + 1) * 512],
                                start=(k == 0), stop=False,
                            )
                        nc.tensor.matmul(
                            ps[0:rws, hn * 512:(hn + 1) * 512],
                            ones1_sb[0:1, 0:rws],
                            bv_sb[0:1, hn * 512:(hn + 1) * 512],
                            start=False, stop=True,
                        )
                    nc.scalar.activation(V_sb[0:rws, g, :], ps[0:rws, :],
                                         mybir.ActivationFunctionType.Copy)

        # ================= phase D: QT[h, t] = ((qh @ Wq.T + bq)/sqrt(H)).T =================
        with ExitStack() as dctx:
            dw = dctx.enter_context(tc.tile_pool(name="ph_d", bufs=1))
            WqTs_sb = dw.tile([P, HT, H], fp32, tag="WqTs")
            nc.sync.dma_start(WqTs_sb[:], part_tiles(WqTs_d[:]))
            dch = dctx.enter_context(tc.tile_pool(name="ph_d_ch", bufs=2))
            dps = dctx.enter_context(tc.tile_pool(name="ph_d_ps", bufs=4, space="PSUM"))
            dbuf = dctx.enter_context(tc.tile_pool(name="ph_d_buf", bufs=4))
            for tn in range(T // 512):
                t0 = tn * 512
                ch = dch.tile([P, HT, 512], fp32, tag="qhch")
                nc.sync.dma_start(
                    ch[:],
                    qhT_d[:, t0:t0 + 512].rearrange("(a p) t -> p a t", p=P))
                for m in range(HT):
                    ps = dps.tile([P, 512], fp32, tag="ps")
                    for k in range(HT):
                        nc.tensor.matmul(
                            ps[:],
                            WqTs_sb[:, k, m * P:(m + 1) * P],
                            ch[:, k, :],
                            start=(k == 0), stop=(k == HT - 1),
                        )
                    buf = dbuf.tile([P, 512], fp32, tag="buf")
                    nc.scalar.activation(buf[:], ps[:],
                                         mybir.ActivationFunctionType.Identity,
                                         bias=bq_sl(m))
                    nc.sync.dma_start(QT_dd[m * P:(m + 1) * P, t0:t0 + 512], buf[:])

        # ================= phase E: per 128-token tile =================
        with ExitStack() as ectx:
            eq = ectx.enter_context(tc.tile_pool(name="e_qt", bufs=2))
            es = ectx.enter_context(tc.tile_pool(name="e_s", bufs=2))
            esm = ectx.enter_context(tc.tile_pool(name="e_smut", bufs=2))
            ev = ectx.enter_context(tc.tile_pool(name="e_vals", bufs=2))
            eat = ectx.enter_context(tc.tile_pool(name="e_attnT", bufs=2))
            eo = ectx.enter_context(tc.tile_pool(name="e_out", bufs=2))
            sc_ps_pool = ectx.enter_context(tc.tile_pool(name="e_sc_ps", bufs=1, space="PSUM"))
            tp_ps_pool = ectx.enter_context(tc.tile_pool(name="e_tp_ps", bufs=2, space="PSUM"))
            u_ps_pool = ectx.enter_context(tc.tile_pool(name="e_u_ps", bufs=1, space="PSUM"))

            for tt in range(TT):
                t0 = tt * P
                # -- load QT k-tiles for this token tile: [128, 8*128]
                qt = eq.tile([P, HT, P], fp32, tag="qt")
                nc.sync.dma_start(
                    qt[:],
                    QT_dd[:, t0:t0 + P].rearrange("(a p) t -> p a t", p=P))

                # -- scores + mask bias into PSUM [128 t, 2016 r]
                sc_ps = sc_ps_pool.tile([P, 2048], fp32, tag="sc")
                for (r0, rw) in R_SLICES:
                    for k in range(HT):
                        nc.tensor.matmul(
                            sc_ps[:, r0:r0 + rw],
                            qt[:, k, :],
                            KT_sb[:, k, r0:r0 + rw],
                            start=(k == 0), stop=False,
                        )
                    nc.tensor.matmul(
                        sc_ps[:, r0:r0 + rw],
                        smf1T_sb[:, t0:t0 + P],
                        maskrhs_sb[:, r0:r0 + rw],
                        start=False, stop=True,
                    )

                # -- evacuate scores to SBUF (ACT)
                s = es.tile([P, R], fp32, tag="s")
                for (r0, rw) in R_SLICES:
                    nc.scalar.activation(s[:, r0:r0 + rw], sc_ps[:, r0:r0 + rw],
                                         mybir.ActivationFunctionType.Copy)

                # -- top-28 threshold via 4 rounds of max8 + match_replace
                vals = ev.tile([P, 32], fp32, tag="vals")
                smut = esm.tile([P, R], fp32, tag="smut")
                nc.vector.max(vals[:, 0:8], s[:])
                nc.vector.match_replace(smut[:], vals[:, 0:8], s[:], NEG_HUGE)
                nc.vector.max(vals[:, 8:16], smut[:])
                nc.vector.match_replace(smut[:], vals[:, 8:16], smut[:], NEG_HUGE)
                nc.vector.max(vals[:, 16:24], smut[:])
                nc.vector.match_replace(smut[:], vals[:, 16:24], smut[:], NEG_HUGE)
                nc.vector.max(vals[:, 24:32], smut[:])
                theta = vals[:, TOP_K - 1:TOP_K]  # 28th largest (desc order)

                # -- prune below-threshold: s += (s < theta) * NEG_HUGE
                _mb = mybir
                nc.vector.tensor_scalar(smut[:], s[:], theta, NEG_HUGE,
                                        op0=_mb.AluOpType.is_lt,
                                        op1=_mb.AluOpType.mult)
                nc.vector.tensor_add(s[:], s[:], smut[:])

                # -- stats: -rowmax, any_act, (later) 1/rowsum
                negm = ev.tile([P, 4], fp32, tag="stats")
                nc.vector.tensor_scalar(negm[:, 0:1], vals[:, 0:1], -1.0, None,
                                        op0=_mb.AluOpType.mult)
                nc.vector.tensor_scalar(negm[:, 1:2], vals[:, 0:1], -BIG / 2.0, None,
                                        op0=_mb.AluOpType.is_gt)

                # -- attn_unnorm = exp(s - rowmax), rowsum fused on ACT
                nc.scalar.activation(smut[:], s[:],
                                     mybir.ActivationFunctionType.Exp,
                                     bias=negm[:, 0:1],
                                     accum_out=negm[:, 2:3])
                nc.vector.reciprocal(negm[:, 3:4], negm[:, 2:3])
                # final scale = any_act / rowsum
                nc.vector.tensor_tensor(negm[:, 3:4], negm[:, 3:4], negm[:, 1:2],
                                        op=_mb.AluOpType.mult)

                # -- transpose attn [t,r] -> attnT [r,t] (bf16), 4 per psum bank-group
                attnT = eat.tile([P, RT, P], bf16, tag="attnT")
                for g in range(4):
                    tp_ps = tp_ps_pool.tile([P, 4, P], fp32, tag="tp")
                    for j in range(4):
                        q = g * 4 + j
                        q0, qw = R_TILES[q]
                        nc.tensor.transpose(tp_ps[0:qw, j, :],
                                            smut[:, q0:q0 + qw],
                                            ident_sb[:])
                    if g < 3:
                        nc.scalar.activation(attnT[:, g * 4:(g + 1) * 4, :],
                                             tp_ps[:],
                                             mybir.ActivationFunctionType.Copy)
                    else:
                        nc.scalar.activation(attnT[:, 12:15, :],
                                             tp_ps[:, 0:3, :],
                                             mybir.ActivationFunctionType.Copy)
                        nc.scalar.activation(attnT[0:96, 15, :],
                                             tp_ps[0:96, 3, :],
                                             mybir.ActivationFunctionType.Copy)

                # -- U = attn @ V  (contract r), normalize+gate on evac
                u_ps = u_ps_pool.tile([P, H], fp32, tag="u")
                for hn in range(2):
                    for q in range(RT):
                        q0, qw = R_TILES[q]
                        nc.tensor.matmul(
                            u_ps[:, hn * 512:(hn + 1) * 512],
                            attnT[0:qw, q, :],
                            V_sb[0:qw, q, hn * 512:(hn + 1) * 512],
                            start=(q == 0), stop=(q == RT - 1),
                        )
                outb = eo.tile([P, H], fp32, tag="outb")
                nc.scalar.activation(outb[:], u_ps[:],
                                     mybir.ActivationFunctionType.Copy,
                                     scale=negm[:, 3:4])
                nc.sync.dma_start(out_d[t0:t0 + P, :], outb[:])

    _split_excess_waits(nc)
    return nc


def _split_excess_waits(nc):
    """TRN2 allows at most 1 semaphore wait per instruction (2 for
    InstEventSemaphore). Tile can emit more; spill the excess onto
    same-engine NoOps inserted just before the instruction."""
    import concourse.mybir as mybir
    import bass_rust

    wid = 0
    for f in nc.m.functions:
        for blk in f.blocks:
            il = blk.instructions
            out = []
            for inst in il:
                si = inst.sync_info
                waits = list(si.on_wait) if si is not None and si.on_wait else []
                limit = 2 if isinstance(inst, mybir.InstEventSemaphore) else 1
                if len(waits) > limit:
                    spill, keep = waits[:-limit], waits[-limit:]
                    for w in spill:
                        nop = mybir.InstNoOp(name=f"WSPILL-{wid}", ins=[], outs=[])
                        wid += 1
                        nop.engine = inst.engine
                        nop.sync_info = bass_rust.SyncInfo(on_wait=[w], on_update=[])
                        out.append(nop)
                    si.on_wait = keep
                    inst.sync_info = si
                out.append(inst)
            if len(out) != len(il):
                il[:] = out


def _host_prep(inputs):
    qh = np.asarray(inputs["query_hidden"], dtype=np.float32)
    sm = np.asarray(inputs["surviving_mask"])
    rel = np.asarray(inputs["rel_embs"], dtype=np.float32)
    f_i = np.asarray(inputs["f_i"]).astype(np.int64)
    f_j = np.asarray(inputs["f_j"]).astype(np.int64)

    scale = 1.0 / math.sqrt(H)

    maskrhs = np.zeros((F + 1, R), dtype=np.float32)
    cols = np.arange(R)
    np.add.at(maskrhs, (f_i, cols), BIG)
    np.add.at(maskrhs, (f_j, cols), BIG)
    maskrhs[F, :] = -2.0 * BIG

    shared = {
        "maskrhs": maskrhs,
        "relT": np.ascontiguousarray(rel.T),
        "WtT": np.ascontiguousarray(np.asarray(inputs["Wt"], np.float32).T),
        "WkT": np.ascontiguousarray(np.asarray(inputs["Wk"], np.float32).T),
        "WvT": np.ascontiguousarray(np.asarray(inputs["Wv"], np.float32).T),
        "WqTs": np.ascontiguousarray(np.asarray(inputs["Wq"], np.float32).T * scale),
        "bt": np.asarray(inputs["bt"], np.float32),
        "bk": np.asarray(inputs["bk"], np.float32),
        "bv": np.asarray(inputs["bv"], np.float32),
        "bqs": np.asarray(inputs["bq"], np.float32) * scale,
    }
    in_maps = []
    for c in range(N_CORES):
        smf1T = np.ones((F + 1, T), dtype=np.float32)
        smf1T[:F, :] = sm[c].T.astype(np.float32)
        m = dict(shared)
        m["qhT"] = np.ascontiguousarray(qh[c].T)
        m["smf1T"] = smf1T
        in_maps.append(m)
    return in_maps


def kernel(**inputs):
    from concourse.bass_utils import run_bass_kernel_spmd

    if "nc" not in _CACHE:
        _CACHE["nc"] = _build_program()
    nc = _CACHE["nc"]

    in_maps = _host_prep(inputs)
    res = run_bass_kernel_spmd(nc, in_maps, list(range(N_CORES)))
    _CACHE["last_results"] = res
    out = np.stack([np.asarray(res.results[c]["out"]) for c in range(N_CORES)])
    return out


# revision 11
# speedup vs baseline: 2.3633x; 2.3633x over previous
"""Trainium2 Bass kernel for nn_DynamicKnowledgeInjector.

Reference computation (per batch b, token t):
    rel_mask = surviving_mask[..., f_i] & surviving_mask[..., f_j]   [B,T,R]
    ta = rel_embs @ Wt.T + bt                                        [R,H]
    Q  = qh @ Wq.T + bq ;  K = ta @ Wk.T + bk ;  V = ta @ Wv.T + bv
    scores = Q @ K.T / sqrt(H), masked to -inf where !rel_mask
    top-28 sparsify -> softmax -> out = attn @ V  (zero row if no active rel)

Sharding: data-parallel over batch; core c owns batch c. Relation-table
work (ta/K/V) is replicated on every core; no collectives.

Device layouts (all activations [feature, token]):
    qhT[H,T], QT[H,T], taT[H,R], KT[H,R] ; V kept natural [R,H] (bf16)
    scores computed [t, r] per 128-token tile, top-k along free dim,
    attn PE-transposed to [r, t] tiles for the AV matmul.

Masking: scores matmul gets a 9th contraction tile of 65 rows:
    lhsT rows = [surviving_mask.T (0/1) ; ones] for the token tile,
    rhs  rows = [BIG*(onehot(f_i)+onehot(f_j)) ; -2*BIG * ones]
so the accumulated bias is BIG*(m_i+m_j-2): exactly 0 for active pairs
(16384+16384-32768 is exact in fp32), -BIG or -2*BIG otherwise. exp()
then underflows those to exactly 0, matching the -inf reference.
"""

import math

import numpy as np

B, T, H, E, F, TOP_K = 8, 2048, 1024, 768, 64, 28
R = 2016
P = 128
BIG = 16384.0  # power of two: mask bias arithmetic is exact in fp32
NEG_HUGE = -1.0e30

N_CORES = 8
HT = H // P   # 8  h-tiles
ET = E // P   # 6  e-tiles
TT = T // P   # 16 t-tiles
# r split into column slices that each fit one PSUM bank (<=512 fp32)
R_SLICES = [(0, 512), (512, 512), (1024, 512), (1536, 480)]
# r split into 128-row contraction tiles for the AV matmul (last is 96)
RT = (R + P - 1) // P  # 16
R_TILES = [(i * P, min(P, R - i * P)) for i in range(RT)]

_CACHE = {}


def _build_program():
    import concourse.bass as bass
    import concourse.mybir as mybir
    from contextlib import ExitStack
    from concourse.tile import TileContext
    from concourse.masks import make_identity

    fp32 = mybir.dt.float32
    bf16 = mybir.dt.bfloat16
    f32r = mybir.dt.float32r

    nc = bass.Bass()

    # ---------------- DRAM parameters ----------------
    qhT_d = nc.declare_dram_parameter("qhT", [H, T], f32r, isOutput=False)
    smf1T_d = nc.declare_dram_parameter("smf1T", [F + 1, T], f32r, isOutput=False)
    maskrhs_d = nc.declare_dram_parameter("maskrhs", [F + 1, R], f32r, isOutput=False)
    relT_d = nc.declare_dram_parameter("relT", [E, R], f32r, isOutput=False)
    WtT_d = nc.declare_dram_parameter("WtT", [E, H], f32r, isOutput=False)
    WkT_d = nc.declare_dram_parameter("WkT", [H, H], f32r, isOutput=False)
    WvT_d = nc.declare_dram_parameter("WvT", [H, H], f32r, isOutput=False)
    WqTs_d = nc.declare_dram_parameter("WqTs", [H, H], f32r, isOutput=False)
    bt_d = nc.declare_dram_parameter("bt", [H], fp32, isOutput=False)
    bk_d = nc.declare_dram_parameter("bk", [H], fp32, isOutput=False)
    bv_d = nc.declare_dram_parameter("bv", [H], f32r, isOutput=False)
    bqs_d = nc.declare_dram_parameter("bqs", [H], fp32, isOutput=False)
    ones1_d = nc.declare_dram_parameter("ones1", [1, P], f32r, isOutput=False)
    out_d = nc.declare_dram_parameter("out", [T, H], fp32, isOutput=True)

    def part_tiles(ap_2d, p=P):
        # [A*p, N] dram view -> [p, A, N] (partition-major tiling of rows)
        return ap_2d.rearrange("(a p) n -> p a n", p=p)

    with TileContext(nc) as tc, ExitStack() as ctx:
        # ------------- resident pools -------------
        res_pool = ctx.enter_context(tc.tile_pool(name="resident", bufs=1))
        KT_sb = res_pool.tile([P, HT, R], f32r, tag="KT")      # [h_loc, ht*R + r] 8MB
        V_sb = res_pool.tile([P, RT, H], bf16, tag="V")        # [r_loc, rt*H + h] 4MB
        smf1T_sb = res_pool.tile([F + 1, T], f32r, tag="smf")
        maskrhs_sb = res_pool.tile([F + 1, R], f32r, tag="mrhs")
        ident_sb = res_pool.tile([P, P], bf16, tag="ident")
        ones1_sb = res_pool.tile([1, P], f32r, tag="ones1")
        bv_sb = res_pool.tile([1, H], f32r, tag="bv")
        bias_sb = res_pool.tile([P, 3 * HT], fp32, tag="biases")  # bt|bk|bqs

        nc.sync.dma_start(smf1T_sb[:], smf1T_d[:])
        nc.sync.dma_start(maskrhs_sb[:], maskrhs_d[:])
        nc.sync.dma_start(bv_sb[:], bv_d[None, :])
        nc.sync.dma_start(bias_sb[:, 0:HT], bt_d[:].rearrange("(a p) -> p a", p=P))
        nc.sync.dma_start(bias_sb[:, HT:2 * HT], bk_d[:].rearrange("(a p) -> p a", p=P))
        nc.sync.dma_start(bias_sb[:, 2 * HT:3 * HT], bqs_d[:].rearrange("(a p) -> p a", p=P))
        nc.sync.dma_start(ones1_sb[:], ones1_d[:])
        make_identity(nc, ident_sb[:])
        bt_sl = lambda m: bias_sb[:, m:m + 1]
        bk_sl = lambda m: bias_sb[:, HT + m:HT + m + 1]
        bq_sl = lambda m: bias_sb[:, 2 * HT + m:2 * HT + m + 1]

        # internal DRAM spill for taT and QT
        dram_pool = ctx.enter_context(tc.tile_pool(name="dram", bufs=1, space="DRAM"))
        taT_dd = dram_pool.tile([H, R], f32r, tag="taT")
        QT_dd = dram_pool.tile([H, T], f32r, tag="QT")

        # ================= phase A: taT[h', r] = (rel_embs @ Wt.T + bt).T =================
        with ExitStack() as actx:
            aw = actx.enter_context(tc.tile_pool(name="ph_a", bufs=1))
            WtT_sb = aw.tile([P, ET, H], f32r, tag="WtT")
            relT_sb = aw.tile([P, ET, R], f32r, tag="relT")
            nc.sync.dma_start(WtT_sb[:], part_tiles(WtT_d[:]))
            nc.sync.dma_start(relT_sb[:], part_tiles(relT_d[:]))
            aps = actx.enter_context(tc.tile_pool(name="ph_a_ps", bufs=4, space="PSUM"))
            abuf = actx.enter_context(tc.tile_pool(name="ph_a_buf", bufs=4))
            for m in range(HT):
                for (r0, rw) in R_SLICES:
                    ps = aps.tile([P, 512], fp32, tag="ps")
                    for k in range(ET):
                        nc.tensor.matmul(
                            ps[:, 0:rw],
                            WtT_sb[:, k, m * P:(m + 1) * P],
                            relT_sb[:, k, r0:r0 + rw],
                            start=(k == 0), stop=(k == ET - 1),
                        )
                    buf = abuf.tile([P, 512], f32r, tag="buf")
                    nc.scalar.activation(buf[:, 0:rw], ps[:, 0:rw],
                                         mybir.ActivationFunctionType.Identity,
                                         bias=bt_sl(m))
                    nc.sync.dma_start(taT_dd[m * P:(m + 1) * P, r0:r0 + rw], buf[:, 0:rw])

        # ================= phase B: KT[h, r] = (ta @ Wk.T + bk).T =================
        with ExitStack() as bctx:
            bw = bctx.enter_context(tc.tile_pool(name="ph_b", bufs=1))
            WkT_sb = bw.tile([P, HT, H], f32r, tag="WkT")
            nc.sync.dma_start(WkT_sb[:], part_tiles(WkT_d[:]))
            bch = bctx.enter_context(tc.tile_pool(name="ph_b_ch", bufs=2))
            bps = bctx.enter_context(tc.tile_pool(name="ph_b_ps", bufs=4, space="PSUM"))
            for (r0, rw) in R_SLICES:
                ch = bch.tile([P, HT, 512], f32r, tag="tach")
                nc.sync.dma_start(
                    ch[:, :, 0:rw],
                    taT_dd[:, r0:r0 + rw].rearrange("(a p) r -> p a r", p=P))
                for m in range(HT):
                    ps = bps.tile([P, 512], fp32, tag="ps")
                    for k in range(HT):
                        nc.tensor.matmul(
                            ps[:, 0:rw],
                            WkT_sb[:, k, m * P:(m + 1) * P],
                            ch[:, k, 0:rw],
                            start=(k == 0), stop=(k == HT - 1),
                        )
                    nc.scalar.activation(KT_sb[:, m, r0:r0 + rw],
                                         ps[:, 0:rw],
                                         mybir.ActivationFunctionType.Identity,
                                         bias=bk_sl(m))

        # ================= phase C: V[r, h] = ta @ Wv.T + bv (bf16) =================
        with ExitStack() as cctx:
            cw = cctx.enter_context(tc.tile_pool(name="ph_c", bufs=1))
            WvT_sb = cw.tile([P, HT, H], f32r, tag="WvT")
            nc.sync.dma_start(WvT_sb[:], part_tiles(WvT_d[:]))
            cch = cctx.enter_context(tc.tile_pool(name="ph_c_ch", bufs=2))
            cps = cctx.enter_context(tc.tile_pool(name="ph_c_ps", bufs=2, space="PSUM"))
            for ri, (r0, rw) in enumerate(R_SLICES):
                ch = cch.tile([P, HT, 512], f32r, tag="tach")
                nc.sync.dma_start(
                    ch[:, :, 0:rw],
                    taT_dd[:, r0:r0 + rw].rearrange("(a p) r -> p a r", p=P))
                for rm in range((rw + P - 1) // P):
                    g = ri * 4 + rm            # global 128-row r-tile index
                    rws = min(P, rw - rm * P)  # 128 or 96
                    ps = cps.tile([P, H], fp32, tag="ps")
                    for hn in range(2):
                        for k in range(HT):
                            nc.tensor.matmul(
                                ps[0:rws, hn * 512:(hn + 1) * 512],
                                ch[:, k, rm * P: rm * P + rws],
                                WvT_sb[:, k, hn * 512:(hn + 1) * 512],
                                start=(k == 0), stop=False,
                            )
                        nc.tensor.matmul(
                            ps[0:rws, hn * 512:(hn + 1) * 512],
                            ones1_sb[0:1, 0:rws],
                            bv_sb[0:1, hn * 512:(hn + 1) * 512],
                            start=False, stop=True,
                        )
                    nc.scalar.activation(V_sb[0:rws, g, :], ps[0:rws, :],
                                         mybir.ActivationFunctionType.Copy)

        # ================= phase D: QT[h, t] = ((qh @ Wq.T + bq)/sqrt(H)).T =================
        with ExitStack() as dctx:
            dw = dctx.enter_context(tc.tile_pool(name="ph_d", bufs=1))
            WqTs_sb = dw.tile([P, HT, H], f32r, tag="WqTs")
            nc.sync.dma_start(WqTs_sb[:], part_tiles(WqTs_d[:]))
            dch = dctx.enter_context(tc.tile_pool(name="ph_d_ch", bufs=2))
            dps = dctx.enter_context(tc.tile_pool(name="ph_d_ps", bufs=4, space="PSUM"))
            dbuf = dctx.enter_context(tc.tile_pool(name="ph_d_buf", bufs=4))
            for tn in range(T // 512):
                t0 = tn * 512
                ch = dch.tile([P, HT, 512], f32r, tag="qhch")
                nc.sync.dma_start(
                    ch[:],
                    qhT_d[:, t0:t0 + 512].rearrange("(a p) t -> p a t", p=P))
                for m in range(HT):
                    ps = dps.tile([P, 512], fp32, tag="ps")
                    for k in range(HT):
                        nc.tensor.matmul(
                            ps[:],
                            WqTs_sb[:, k, m * P:(m + 1) * P],
                            ch[:, k, :],
                            start=(k == 0), stop=(k == HT - 1),
                        )
                    buf = dbuf.tile([P, 512], f32r, tag="buf")
                    nc.scalar.activation(buf[:], ps[:],
                                         mybir.ActivationFunctionType.Identity,
                                         bias=bq_sl(m))
                    nc.sync.dma_start(QT_dd[m * P:(m + 1) * P, t0:t0 + 512], buf[:])

        # ================= phase E: per 128-token tile =================
        with ExitStack() as ectx:
            eq = ectx.enter_context(tc.tile_pool(name="e_qt", bufs=2))
            es = ectx.enter_context(tc.tile_pool(name="e_s", bufs=2))
            esm = ectx.enter_context(tc.tile_pool(name="e_smut", bufs=2))
            ee = ectx.enter_context(tc.tile_pool(name="e_exp", bufs=2))
            ev = ectx.enter_context(tc.tile_pool(name="e_vals", bufs=2))
            eat = ectx.enter_context(tc.tile_pool(name="e_attnT", bufs=2))
            eo = ectx.enter_context(tc.tile_pool(name="e_out", bufs=2))
            sc_ps_pool = ectx.enter_context(tc.tile_pool(name="e_sc_ps", bufs=1, space="PSUM"))
            tp_ps_pool = ectx.enter_context(tc.tile_pool(name="e_tp_ps", bufs=2, space="PSUM"))
            u_ps_pool = ectx.enter_context(tc.tile_pool(name="e_u_ps", bufs=1, space="PSUM"))

            for tt in range(TT):
                t0 = tt * P
                # -- load QT k-tiles for this token tile: [128, 8*128]
                qt = eq.tile([P, HT, P], f32r, tag="qt")
                nc.sync.dma_start(
                    qt[:],
                    QT_dd[:, t0:t0 + P].rearrange("(a p) t -> p a t", p=P))

                # -- scores + mask bias into PSUM [128 t, 2016 r]
                sc_ps = sc_ps_pool.tile([P, 2048], fp32, tag="sc")
                for (r0, rw) in R_SLICES:
                    for k in range(HT):
                        nc.tensor.matmul(
                            sc_ps[:, r0:r0 + rw],
                            qt[:, k, :],
                            KT_sb[:, k, r0:r0 + rw],
                            start=(k == 0), stop=False,
                        )
                    nc.tensor.matmul(
                        sc_ps[:, r0:r0 + rw],
                        smf1T_sb[:, t0:t0 + P],
                        maskrhs_sb[:, r0:r0 + rw],
                        start=False, stop=True,
                    )

                # -- evacuate scores to SBUF (ACT)
                s = es.tile([P, R], fp32, tag="s")
                for (r0, rw) in R_SLICES:
                    nc.scalar.activation(s[:, r0:r0 + rw], sc_ps[:, r0:r0 + rw],
                                         mybir.ActivationFunctionType.Copy)

                # -- top-28 threshold via 4 rounds of max8 + match_replace
                vals = ev.tile([P, 32], fp32, tag="vals")
                smut = esm.tile([P, R], fp32, tag="smut")
                nc.vector.max(vals[:, 0:8], s[:])
                nc.vector.match_replace(smut[:], vals[:, 0:8], s[:], NEG_HUGE)
                nc.vector.max(vals[:, 8:16], smut[:])
                nc.vector.match_replace(smut[:], vals[:, 8:16], smut[:], NEG_HUGE)
                nc.vector.max(vals[:, 16:24], smut[:])
                nc.vector.match_replace(smut[:], vals[:, 16:24], smut[:], NEG_HUGE)
                nc.vector.max(vals[:, 24:32], smut[:])
                theta = vals[:, TOP_K - 1:TOP_K]  # 28th largest (desc order)

                # -- prune below-threshold: s += (s < theta) * NEG_HUGE
                _mb = mybir
                nc.vector.tensor_scalar(smut[:], s[:], theta, NEG_HUGE,
                                        op0=_mb.AluOpType.is_lt,
                                        op1=_mb.AluOpType.mult)
                nc.vector.tensor_add(s[:], s[:], smut[:])

                # -- stats: -rowmax, any_act, (later) 1/rowsum
                negm = ev.tile([P, 4], fp32, tag="stats")
                nc.vector.tensor_scalar(negm[:, 0:1], vals[:, 0:1], -1.0, None,
                                        op0=_mb.AluOpType.mult)
                nc.vector.tensor_scalar(negm[:, 1:2], vals[:, 0:1], -BIG / 2.0, None,
                                        op0=_mb.AluOpType.is_gt)

                # -- attn_unnorm = exp(s - rowmax) in bf16, rowsum fused on ACT
                e = ee.tile([P, R], bf16, tag="e")
                nc.scalar.activation(e[:], s[:],
                                     mybir.ActivationFunctionType.Exp,
                                     bias=negm[:, 0:1],
                                     accum_out=negm[:, 2:3])
                nc.vector.reciprocal(negm[:, 3:4], negm[:, 2:3])
                # final scale = any_act / rowsum
                nc.vector.tensor_tensor(negm[:, 3:4], negm[:, 3:4], negm[:, 1:2],
                                        op=_mb.AluOpType.mult)

                # -- transpose attn [t,r] -> attnT [r,t] (bf16), 4 per psum bank-group
                attnT = eat.tile([P, RT, P], bf16, tag="attnT")
                for g in range(4):
                    tp_ps = tp_ps_pool.tile([P, 4, P], bf16, tag="tp")
                    for j in range(4):
                        q = g * 4 + j
                        q0, qw = R_TILES[q]
                        nc.tensor.transpose(tp_ps[0:qw, j, :],
                                            e[:, q0:q0 + qw],
                                            ident_sb[:])
                    if g < 3:
                        nc.scalar.activation(attnT[:, g * 4:(g + 1) * 4, :],
                                             tp_ps[:],
                                             mybir.ActivationFunctionType.Copy)
                    else:
                        nc.scalar.activation(attnT[:, 12:15, :],
                                             tp_ps[:, 0:3, :],
                                             mybir.ActivationFunctionType.Copy)
                        nc.scalar.activation(attnT[0:96, 15, :],
                                             tp_ps[0:96, 3, :],
                                             mybir.ActivationFunctionType.Copy)

                # -- U = attn @ V  (contract r), normalize+gate on evac
                u_ps = u_ps_pool.tile([P, H], fp32, tag="u")
                for hn in range(2):
                    for q in range(RT):
                        q0, qw = R_TILES[q]
                        nc.tensor.matmul(
                            u_ps[:, hn * 512:(hn + 1) * 512],
                            attnT[0:qw, q, :],
                            V_sb[0:qw, q, hn * 512:(hn + 1) * 512],
                            start=(q == 0), stop=(q == RT - 1),
                        )
                outb = eo.tile([P, H], fp32, tag="outb")
                nc.scalar.activation(outb[:], u_ps[:],
                                     mybir.ActivationFunctionType.Copy,
                                     scale=negm[:, 3:4])
                nc.sync.dma_start(out_d[t0:t0 + P, :], outb[:])

    _split_excess_waits(nc)
    return nc


def _split_excess_waits(nc):
    """TRN2 allows at most 1 semaphore wait per instruction (2 for
    InstEventSemaphore). Tile can emit more; spill the excess onto
    same-engine NoOps inserted just before the instruction."""
    import concourse.mybir as mybir
    import bass_rust

    wid = 0
    for f in nc.m.functions:
        for blk in f.blocks:
            il = blk.instructions
            out = []
            for inst in il:
                si = inst.sync_info
                waits = list(si.on_wait) if si is not None and si.on_wait else []
                limit = 2 if isinstance(inst, mybir.InstEventSemaphore) else 1
                if len(waits) > limit:
                    spill, keep = waits[:-limit], waits[-limit:]
                    for w in spill:
                        nop = mybir.InstNoOp(name=f"WSPILL-{wid}", ins=[], outs=[])
                        wid += 1
                        nop.engine = inst.engine
                        nop.sync_info = bass_rust.SyncInfo(on_wait=[w], on_update=[])
                        out.append(nop)
                    si.on_wait = keep
                    inst.sync_info = si
                out.append(inst)
            if len(out) != len(il):
                il[:] = out


def _host_prep(inputs):
    qh = np.asarray(inputs["query_hidden"], dtype=np.float32)
    sm = np.asarray(inputs["surviving_mask"])
    rel = np.asarray(inputs["rel_embs"], dtype=np.float32)
    f_i = np.asarray(inputs["f_i"]).astype(np.int64)
    f_j = np.asarray(inputs["f_j"]).astype(np.int64)

    scale = 1.0 / math.sqrt(H)

    maskrhs = np.zeros((F + 1, R), dtype=np.float32)
    cols = np.arange(R)
    np.add.at(maskrhs, (f_i, cols), BIG)
    np.add.at(maskrhs, (f_j, cols), BIG)
    maskrhs[F, :] = -2.0 * BIG

    shared = {
        "maskrhs": maskrhs,
        "relT": np.ascontiguousarray(rel.T),
        "WtT": np.ascontiguousarray(np.asarray(inputs["Wt"], np.float32).T),
        "WkT": np.ascontiguousarray(np.asarray(inputs["Wk"], np.float32).T),
        "WvT": np.ascontiguousarray(np.asarray(inputs["Wv"], np.float32).T),
        "WqTs": np.ascontiguousarray(
            np.asarray(inputs["Wq"], np.float32).T * scale),
        "bt": np.asarray(inputs["bt"], np.float32),
        "bk": np.asarray(inputs["bk"], np.float32),
        "bv": np.asarray(inputs["bv"], np.float32),
        "bqs": np.asarray(inputs["bq"], np.float32) * scale,
        "ones1": np.ones((1, P), np.float32),
    }
    in_maps = []
    for c in range(N_CORES):
        smf1T = np.ones((F + 1, T), dtype=np.float32)
        smf1T[:F, :] = sm[c].T.astype(np.float32)
        m = dict(shared)
        m["qhT"] = np.ascontiguousarray(qh[c].T)
        m["smf1T"] = smf1T
        in_maps.append(m)
    return in_maps


def kernel(**inputs):
    from concourse.bass_utils import run_bass_kernel_spmd

    if "nc" not in _CACHE:
        _CACHE["nc"] = _build_program()
    nc = _CACHE["nc"]

    in_maps = _host_prep(inputs)
    res = run_bass_kernel_spmd(nc, in_maps, list(range(N_CORES)))
    _CACHE["last_results"] = res
    out = np.stack([np.asarray(res.results[c]["out"]) for c in range(N_CORES)])
    return out
